# revision 34
# baseline (speedup 1.0000x reference)
"""MoE layer (8 experts, top-2 sigmoid routing, SwiGLU experts + shared expert)
on 8 TRN2 NeuronCores.

Strategy (expert-parallel, host-side token dispatch):
  - Router (sigmoid(x @ gate_w.T), top-2, weight normalization) is tiny
    (~50 MFLOP) and runs on the host; it determines the all-to-all dispatch.
  - Core c owns expert c: it gets the tokens routed to expert c (gathered and
    zero-padded to a common capacity m_pad) plus expert c's Wi/Wo.
  - The shared expert is data-parallel: core c also processes tokens
    [c*512, (c+1)*512) with the (replicated) shared weights.
  - Device kernel computes the two SwiGLU MLP passes in bf16 (fp32 PSUM
    accumulation), feature-major layout (features on partitions, tokens on the
    free dim) so no on-device transposes are needed.
  - Host combine: out[t] = shared_out[t] + sum_e cw[e,t] * expert_out[e][t]
    (the combine weights are applied on the host during the scatter-add).

Lead-in design (the PE stream is at the bf16 roofline once running, so the
measured time is roofline + lead-in + tail):
  - All inputs are host-pre-permuted to partition-major so every load is a
    few large DMAs (~0.4-1.8 MB) instead of ~60 small ones: the HWDGE issue
    cost (~600 ns per dma_start on the issuing engine) was the real limit on
    how fast the first MB landed, not HBM bandwidth.
  - Loads are split across BOTH HWDGE queues (sync + scalar) in exact
    consumption order; each queue drains FIFO, so the critical first chain's
    data (xs + first swi pair) transfers before anything else.
  - ~10 junk matmuls on an uninitialized tile bridge the PE from the end of
    the fixed NEFF preamble (~6.5us) to the first real chain (~10.5us) and
    un-throttle the HAM clock gate (4/8 -> 8/8) before real work starts.
  - Outputs are written bf16 (host combines in fp32): halves output DMA.
"""

from contextlib import ExitStack

import ml_dtypes
import numpy as np

import concourse.tile as tile
from concourse import bacc, mybir
from concourse.bass_utils import run_bass_kernel_spmd

E, TOPK, H, I = 8, 2, 768, 1152
I2 = 2 * I
T = 4096
N_CORES = 8
TS = T // N_CORES  # shared-expert tokens per core
P = 128
KH = H // P    # 6 contraction tiles over H
KI = I // P    # 9 contraction tiles over I
BF16 = mybir.dt.bfloat16
F32 = mybir.dt.float32
MAXN = 512     # max tokens per matmul chunk (one fp32 PSUM bank)
N_WARM = 9     # junk matmuls bridging preamble -> first real chain

_BUILD_CACHE: dict = {}
LAST_RESULTS = None  # BassKernelResults of the most recent device run
USE_SILU = True  # native ACT Silu on HW; set False for CoreSim (not implemented there)


def _ensure_axon_ntff_hook():
    """This image's `antenv` lacks the `axon_hooks` module that
    run_bass_kernel_spmd imports when NTFF tracing is requested (BASS_TRACE=1).
    Install an equivalent shim so profiling works instead of crashing."""
    try:
        import antenv.axon_hooks  # noqa: F401
        return
    except ImportError:
        pass
    import sys
    import types
    try:
        import antenv
    except ImportError:
        return
    mod = types.ModuleType("antenv.axon_hooks")
    holder = {"hook": None}
    mod.set_axon_ntff_profile_hook = lambda h: holder.__setitem__("hook", h)
    mod.get_axon_ntff_profile_hook = lambda: holder["hook"]
    sys.modules["antenv.axon_hooks"] = mod
    antenv.axon_hooks = mod
    so_path = "/opt/axon/libaxon_pjrt.so"
    try:
        import os
        if os.path.exists(so_path):
            from trn_agent_boot.trn_boot import _ntff_profile_via_ctypes
            hook = _ntff_profile_via_ctypes(so_path)
            if hook is not None:
                mod.set_axon_ntff_profile_hook(hook)
    except Exception:
        pass  # hook stays None; bass_utils logs a warning and skips tracing


def _chunk_sizes(m: int) -> list[int]:
    """Split m into ceil(m/512) near-equal chunks, smallest last."""
    n = -(-m // MAXN)
    base, rem = divmod(m, n)
    return [base + 1] * rem + [base] * (n - rem)


def _build(m_pad: int):
    nc = bacc.Bacc("TRN2", target_bir_lowering=False, debug=False,
                   num_devices=N_CORES)

    # All inputs arrive host-pre-permuted to partition-major: dram[p, kt, f]
    # holds row kt*128+p of the logical (rows, f) matrix, so each SBUF tile
    # loads as one large contiguous-per-partition DMA.
    xs = nc.dram_tensor("xs", [P, KH, TS], BF16, kind="ExternalInput").ap()
    # swi[ft] = [P, 2H]: cols 0:H are the proj-half f-tile ft, H:2H the
    # gate-half f-tile ft (pairs packed in exact PE consumption order).
    swi = nc.dram_tensor("swi", [KI, P, 2 * H], BF16, kind="ExternalInput").ap()
    swo = nc.dram_tensor("swo", [P, KI, H], BF16, kind="ExternalInput").ap()
    xe = nc.dram_tensor("xe", [P, KH, m_pad], BF16, kind="ExternalInput").ap()
    wi = nc.dram_tensor("wi", [P, KH, I2], BF16, kind="ExternalInput").ap()
    wo = nc.dram_tensor("wo", [P, KI, H], BF16, kind="ExternalInput").ap()
    ye = nc.dram_tensor("ye", [H, m_pad], BF16, kind="ExternalOutput").ap()
    ys = nc.dram_tensor("ys", [H, TS], BF16, kind="ExternalOutput").ap()

    with ExitStack() as ctx:
        tc = ctx.enter_context(tile.TileContext(nc))
        wpool = ctx.enter_context(tc.tile_pool(name="weights", bufs=1))
        apool = ctx.enter_context(tc.tile_pool(name="act", bufs=3))
        spool = ctx.enter_context(tc.tile_pool(name="silu", bufs=4))
        ypool = ctx.enter_context(tc.tile_pool(name="y", bufs=3))
        # all 8 PSUM banks in one pool; the warm-up tiles share the "ps" tag
        # so their 2 banks recycle into the working set after the lead-in
        psum = ctx.enter_context(tc.tile_pool(name="psum", bufs=8, space="PSUM"))

        # Junk matmuls fill the otherwise-idle PE during the DMA lead-in: the
        # HAM clock gate sees a busy window and un-throttles (4/8 -> 8/8)
        # before the real matmuls start. The producing memset rides the
        # GpSimd queue (idle, earliest out of the NEFF preamble) so the first
        # junk matmul can issue ~1us earlier than with a DVE memset.
        warm = wpool.tile([P, MAXN], BF16, tag="warm", name="warm")
        nc.gpsimd.memset(warm[:], 0.0)
        wps = [psum.tile([P, MAXN], F32, tag="ps", name=f"wps{i}")
               for i in range(2)]
        for i in range(N_WARM):
            nc.tensor.matmul(wps[i % 2], lhsT=warm[:, :P], rhs=warm[:],
                             start=True, stop=True)

        # ---- input loads: consumption order, alternating HWDGE queues ----
        # ALL input loads ride the sync HWDGE ring, which drains FIFO per
        # issuing engine — so this emission order IS the transfer order, and
        # it is exactly consumption order. The scalar engine carries no input
        # DMAs at all: a dma_start blocks the issuing engine while the ring
        # is full, and the scalar engine must be free to run the SwiGLU silu
        # ops (which drain PSUM banks) from ~13us on — parking multi-MB loads
        # on it stalls the PE behind 8 full PSUM banks.
        def sload(shape, tag, src):
            t = wpool.tile(shape, BF16, tag=tag, name=tag)
            nc.sync.dma_start(t[:], src)
            return t

        # Ring head: ONLY the proj half of swi pair 0 (the first chain's
        # lhsT, 192KB) goes first, then xs as three 2-k-tile DMAs (262KB
        # each: big enough to stay ring-bound instead of issue-bound), then
        # the gate half of pair 0 and the remaining pairs WHOLE (384KB —
        # splitting them all makes the stream issue-bound). Chunk 1's finish
        # is pinned by the supply time of the first chain's critical set
        # (now 978KB instead of 1.17MB); later pairs land with growing
        # margin over their 2.56us/pair consumption.
        swi0a = sload([P, H], "swi0a", swi[0][:, :H])
        xs_t = [sload([P, 2, TS], f"xs{i}", xs[:, 2 * i:2 * i + 2, :])
                for i in range(KH // 2)]
        swi0b = sload([P, H], "swi0b", swi[0][:, H:])
        swi_t = [None] + [sload([P, 2 * H], f"swi{ft}", swi[ft])
                          for ft in range(1, KI)]
        swo_t = sload([P, KI, H], "swo", swo[:])
        xe_t = sload([P, KH, m_pad], "xe", xe[:])
        wi_t = sload([P, KH, I2], "wi", wi[:])
        wo_t = sload([P, KI, H], "wo", wo[:])

        # accessors: x(kt, a, b) -> rhs AP; w*(ft|ht, kt) -> lhsT AP
        sh = dict(
            x=lambda kt, a, b: xs_t[kt // 2][:, kt % 2, a:b],
            wa=lambda ft, kt: (swi0a if ft == 0 else
                               swi_t[ft])[:, kt * P:(kt + 1) * P],
            wb=lambda ft, kt: (swi0b[:, kt * P:(kt + 1) * P] if ft == 0 else
                               swi_t[ft][:, H + kt * P:H + (kt + 1) * P]),
            wo=lambda ht, kt: swo_t[:, kt, ht * P:(ht + 1) * P],
        )
        ex = dict(
            x=lambda kt, a, b: xe_t[:, kt, a:b],
            wa=lambda ft, kt: wi_t[:, kt, ft * P:(ft + 1) * P],
            wb=lambda ft, kt: wi_t[:, kt, I + ft * P:I + (ft + 1) * P],
            wo=lambda ht, kt: wo_t[:, kt, ht * P:(ht + 1) * P],
        )

        # (accessors, y_dram, chunk_off, chunk_sz, silu_on_first)
        chunks = []
        for acc, yd, m, sfirst in ((sh, ys, TS, True), (ex, ye, m_pad, False)):
            off = 0
            for sz in _chunk_sizes(m):
                chunks.append((acc, yd, off, sz, sfirst))
                off += sz

        def emit_wi(c):
            acc, yd, off, sz, sfirst = chunks[c]
            act = apool.tile([P, KI, MAXN], BF16, tag="act", name="act")[:, :, :sz]
            for ft in range(KI):
                ps_a = psum.tile([P, MAXN], F32, tag="ps", name="ps_a")[:, :sz]
                for kt in range(KH):
                    nc.tensor.matmul(ps_a, lhsT=acc["wa"](ft, kt),
                                     rhs=acc["x"](kt, off, off + sz),
                                     start=(kt == 0), stop=(kt == KH - 1))
                ps_b = psum.tile([P, MAXN], F32, tag="ps", name="ps_b")[:, :sz]
                for kt in range(KH):
                    nc.tensor.matmul(ps_b, lhsT=acc["wb"](ft, kt),
                                     rhs=acc["x"](kt, off, off + sz),
                                     start=(kt == 0), stop=(kt == KH - 1))
                sl = spool.tile([P, MAXN], F32, tag="silu", name="sl")[:, :sz]
                ps_s, ps_m = (ps_a, ps_b) if sfirst else (ps_b, ps_a)
                if USE_SILU:
                    # act = silu(s) * m: one ACT op + one DVE mul; PSUM banks
                    # are freed one op earlier than the sigmoid+2-mul form
                    nc.scalar.activation(sl, ps_s,
                                         mybir.ActivationFunctionType.Silu)
                    nc.vector.tensor_mul(act[:, ft, :], sl, ps_m)
                else:
                    # CoreSim fallback: silu(s) = s * sigmoid(s)
                    tmp = spool.tile([P, MAXN], F32, tag="silu2",
                                     name="tmp")[:, :sz]
                    nc.scalar.activation(sl, ps_s,
                                         mybir.ActivationFunctionType.Sigmoid)
                    nc.vector.tensor_mul(tmp, sl, ps_s)
                    nc.vector.tensor_mul(act[:, ft, :], tmp, ps_m)
            return act

        def emit_wo(c, act, last=False):
            acc, yd, off, sz, sfirst = chunks[c]
            for ht in range(KH):
                ps_y = psum.tile([P, MAXN], F32, tag="ps", name="ps_y")[:, :sz]
                for kt in range(KI):
                    nc.tensor.matmul(ps_y, lhsT=acc["wo"](ht, kt),
                                     rhs=act[:, kt, :],
                                     start=(kt == 0), stop=(kt == KI - 1))
                yt = ypool.tile([P, MAXN], BF16, tag="y", name="yt")[:, :sz]
                # copy (with f32->bf16 cast) + output DMA both on the Scalar
                # engine: its HWDGE ring carries no input loads, so outputs
                # drain immediately and never contend with the input stream on
                # the sync ring. The LAST chunk's copies ride the (idle-by-
                # then) Vector engine instead — ~3x faster per copy, pulling
                # the final output DMA earlier in the serialized tail.
                if last:
                    nc.vector.tensor_copy(yt, ps_y)
                else:
                    nc.scalar.copy(yt, ps_y)
                nc.scalar.dma_start(
                    yd.rearrange("(o p) m -> p o m", p=P)[:, ht, off:off + sz], yt)

        # software pipeline: Wi(c+1) is emitted before Wo(c) so the PE always
        # has independent matmul work while ACT/DVE finish chunk c's SwiGLU.
        n = len(chunks)
        acts = [None] * n
        acts[0] = emit_wi(0)
        emit_wo(0, acts[0])
        if n > 1:
            acts[1] = emit_wi(1)
            for c in range(2, n):
                acts[c] = emit_wi(c)
                emit_wo(c - 1, acts[c - 1])
            emit_wo(n - 1, acts[n - 1], last=True)

    nc.compile()
    return nc


def _p_major(a: np.ndarray, ktiles: int) -> np.ndarray:
    """(ktiles*P, F) -> (P, ktiles, F) partition-major contiguous layout."""
    return np.ascontiguousarray(
        a.reshape(ktiles, P, a.shape[1]).transpose(1, 0, 2))


def _tile_swi(swiT):
    """(H, 2I) -> (9, P, 2H): f-tile-pair-major layout, pairs packed in the
    exact order the PE consumes them (proj f-tile ft || gate f-tile ft)."""
    FI2 = I2 // P
    t18 = swiT.reshape(KH, P, FI2, P).transpose(2, 1, 0, 3).reshape(FI2, P, H)
    return np.ascontiguousarray(np.concatenate([t18[:KI], t18[KI:]], axis=2))


def _route(x, gate_w, correction_bias):
    logits = 1.0 / (1.0 + np.exp(-(x @ gate_w.T), dtype=np.float32))  # (T, E)
    sel = logits + correction_bias[None, :]
    order = np.argsort(-sel, axis=1, kind="stable")[:, :TOPK]  # ties -> low index
    w = np.take_along_axis(logits, order, axis=1)
    w = (w / w.sum(axis=1, keepdims=True)).astype(np.float32)
    return order, w


def kernel(**inputs) -> np.ndarray:
    x = np.asarray(inputs["x"], np.float32)
    gate_w = np.asarray(inputs["gate_w"], np.float32)
    bias = np.asarray(inputs["correction_bias"], np.float32)
    Wi = np.asarray(inputs["Wi"], np.float32)
    Wo = np.asarray(inputs["Wo"], np.float32)
    shared_Wi = np.asarray(inputs["shared_Wi"], np.float32)
    shared_Wo = np.asarray(inputs["shared_Wo"], np.float32)

    order, w = _route(x, gate_w, bias)

    idx_per_e, cw_per_e = [], []
    for e in range(E):
        mask = order == e  # (T, K)
        tok = mask.any(axis=1)
        rows = np.nonzero(tok)[0]
        kpos = np.argmax(mask[rows], axis=1)
        idx_per_e.append(rows)
        cw_per_e.append(w[rows, kpos].astype(np.float32))

    mx = max(len(r) for r in idx_per_e)
    m_pad = max(64, mx + (mx & 1))  # exact capacity, kept even for alignment

    bf = ml_dtypes.bfloat16
    xT = np.ascontiguousarray(x.T).astype(bf)            # (H, T)
    swip = _tile_swi(shared_Wi.T.astype(bf))             # (9, P, 2H)
    swop = _p_major(np.ascontiguousarray(shared_Wo.T).astype(bf), KI)

    in_maps = []
    for c in range(N_CORES):
        rows = idx_per_e[c]
        xe = np.zeros((H, m_pad), bf)
        xe[:, :len(rows)] = xT[:, rows]
        in_maps.append({
            "xs": _p_major(np.ascontiguousarray(
                xT[:, c * TS:(c + 1) * TS]), KH),         # (P, 6, TS)
            "swi": swip,
            "swo": swop,
            "xe": _p_major(xe, KH),                       # (P, 6, m_pad)
            "wi": _p_major(Wi[c].astype(bf), KH),         # (P, 6, 2I)
            "wo": _p_major(Wo[c].astype(bf), KI),         # (P, 9, H)
        })

    if m_pad not in _BUILD_CACHE:
        _BUILD_CACHE[m_pad] = _build(m_pad)
    nc = _BUILD_CACHE[m_pad]

    _ensure_axon_ntff_hook()
    res = run_bass_kernel_spmd(nc, in_maps, list(range(N_CORES)))
    global LAST_RESULTS
    LAST_RESULTS = res

    out = np.zeros((T, H), np.float32)
    for c in range(N_CORES):
        r = res.results[c]
        out[c * TS:(c + 1) * TS] += r["ys"].T.astype(np.float32)
        rows = idx_per_e[c]
        if len(rows):
            out[rows] += (r["ye"][:, :len(rows)].T.astype(np.float32)
                          * cw_per_e[c][:, None])
    return out


# revision 35
# speedup vs baseline: 1.0061x; 1.0061x over previous
"""MoE layer (8 experts, top-2 sigmoid routing, SwiGLU experts + shared expert)
on 8 TRN2 NeuronCores.

Strategy (expert-parallel, host-side token dispatch):
  - Router (sigmoid(x @ gate_w.T), top-2, weight normalization) is tiny
    (~50 MFLOP) and runs on the host; it determines the all-to-all dispatch.
  - Core c owns expert c: it gets the tokens routed to expert c (gathered and
    zero-padded to a common capacity m_pad) plus expert c's Wi/Wo.
  - The shared expert is data-parallel: core c also processes tokens
    [c*512, (c+1)*512) with the (replicated) shared weights.
  - Device kernel computes the two SwiGLU MLP passes in bf16 (fp32 PSUM
    accumulation), feature-major layout (features on partitions, tokens on the
    free dim) so no on-device transposes are needed.
  - Host combine: out[t] = shared_out[t] + sum_e cw[e,t] * expert_out[e][t]
    (the combine weights are applied on the host during the scatter-add).

Lead-in design (the PE stream is at the bf16 roofline once running, so the
measured time is roofline + lead-in + tail):
  - All inputs are host-pre-permuted to partition-major so every load is a
    few large DMAs (~0.4-1.8 MB) instead of ~60 small ones: the HWDGE issue
    cost (~600 ns per dma_start on the issuing engine) was the real limit on
    how fast the first MB landed, not HBM bandwidth.
  - Loads are split across BOTH HWDGE queues (sync + scalar) in exact
    consumption order; each queue drains FIFO, so the critical first chain's
    data (xs + first swi pair) transfers before anything else.
  - ~10 junk matmuls on an uninitialized tile bridge the PE from the end of
    the fixed NEFF preamble (~6.5us) to the first real chain (~10.5us) and
    un-throttle the HAM clock gate (4/8 -> 8/8) before real work starts.
  - Outputs are written bf16 (host combines in fp32): halves output DMA.
"""

from contextlib import ExitStack

import ml_dtypes
import numpy as np

import concourse.tile as tile
from concourse import bacc, mybir
from concourse.bass_utils import run_bass_kernel_spmd

E, TOPK, H, I = 8, 2, 768, 1152
I2 = 2 * I
T = 4096
N_CORES = 8
TS = T // N_CORES  # shared-expert tokens per core
P = 128
KH = H // P    # 6 contraction tiles over H
KI = I // P    # 9 contraction tiles over I
BF16 = mybir.dt.bfloat16
F32 = mybir.dt.float32
MAXN = 512     # max tokens per matmul chunk (one fp32 PSUM bank)
N_WARM = 9     # junk matmuls bridging preamble -> first real chain

_BUILD_CACHE: dict = {}
LAST_RESULTS = None  # BassKernelResults of the most recent device run
USE_SILU = True  # native ACT Silu on HW; set False for CoreSim (not implemented there)


def _ensure_axon_ntff_hook():
    """This image's `antenv` lacks the `axon_hooks` module that
    run_bass_kernel_spmd imports when NTFF tracing is requested (BASS_TRACE=1).
    Install an equivalent shim so profiling works instead of crashing."""
    try:
        import antenv.axon_hooks  # noqa: F401
        return
    except ImportError:
        pass
    import sys
    import types
    try:
        import antenv
    except ImportError:
        return
    mod = types.ModuleType("antenv.axon_hooks")
    holder = {"hook": None}
    mod.set_axon_ntff_profile_hook = lambda h: holder.__setitem__("hook", h)
    mod.get_axon_ntff_profile_hook = lambda: holder["hook"]
    sys.modules["antenv.axon_hooks"] = mod
    antenv.axon_hooks = mod
    so_path = "/opt/axon/libaxon_pjrt.so"
    try:
        import os
        if os.path.exists(so_path):
            from trn_agent_boot.trn_boot import _ntff_profile_via_ctypes
            hook = _ntff_profile_via_ctypes(so_path)
            if hook is not None:
                mod.set_axon_ntff_profile_hook(hook)
    except Exception:
        pass  # hook stays None; bass_utils logs a warning and skips tracing


def _chunk_sizes(m: int) -> list[int]:
    """Split m into ceil(m/512) near-equal chunks, smallest last."""
    n = -(-m // MAXN)
    base, rem = divmod(m, n)
    return [base + 1] * rem + [base] * (n - rem)


def _build(m_pad: int):
    nc = bacc.Bacc("TRN2", target_bir_lowering=False, debug=False,
                   num_devices=N_CORES)

    # All inputs arrive host-pre-permuted to partition-major: dram[p, kt, f]
    # holds row kt*128+p of the logical (rows, f) matrix, so each SBUF tile
    # loads as one large contiguous-per-partition DMA.
    xs = nc.dram_tensor("xs", [P, KH, TS], BF16, kind="ExternalInput").ap()
    # swi[ft] = [P, 2H]: cols 0:H are the proj-half f-tile ft, H:2H the
    # gate-half f-tile ft (pairs packed in exact PE consumption order).
    swi = nc.dram_tensor("swi", [KI, P, 2 * H], BF16, kind="ExternalInput").ap()
    swo = nc.dram_tensor("swo", [P, KI, H], BF16, kind="ExternalInput").ap()
    xe = nc.dram_tensor("xe", [P, KH, m_pad], BF16, kind="ExternalInput").ap()
    wi = nc.dram_tensor("wi", [P, KH, I2], BF16, kind="ExternalInput").ap()
    wo = nc.dram_tensor("wo", [P, KI, H], BF16, kind="ExternalInput").ap()
    ye = nc.dram_tensor("ye", [H, m_pad], BF16, kind="ExternalOutput").ap()
    ys = nc.dram_tensor("ys", [H, TS], BF16, kind="ExternalOutput").ap()

    with ExitStack() as ctx:
        tc = ctx.enter_context(tile.TileContext(nc))
        wpool = ctx.enter_context(tc.tile_pool(name="weights", bufs=1))
        apool = ctx.enter_context(tc.tile_pool(name="act", bufs=3))
        spool = ctx.enter_context(tc.tile_pool(name="silu", bufs=4))
        ypool = ctx.enter_context(tc.tile_pool(name="y", bufs=3))
        # all 8 PSUM banks in one pool; the warm-up tiles share the "ps" tag
        # so their 2 banks recycle into the working set after the lead-in
        psum = ctx.enter_context(tc.tile_pool(name="psum", bufs=8, space="PSUM"))

        # Junk matmuls fill the otherwise-idle PE during the DMA lead-in: the
        # HAM clock gate sees a busy window and un-throttles (4/8 -> 8/8)
        # before the real matmuls start. The producing memset rides the
        # GpSimd queue (idle, earliest out of the NEFF preamble) so the first
        # junk matmul can issue ~1us earlier than with a DVE memset.
        warm = wpool.tile([P, MAXN], BF16, tag="warm", name="warm")
        nc.gpsimd.memset(warm[:], 0.0)
        wps = [psum.tile([P, MAXN], F32, tag="ps", name=f"wps{i}")
               for i in range(2)]
        for i in range(N_WARM):
            nc.tensor.matmul(wps[i % 2], lhsT=warm[:, :P], rhs=warm[:],
                             start=True, stop=True)

        # ---- input loads: consumption order, alternating HWDGE queues ----
        # ALL input loads ride the sync HWDGE ring, which drains FIFO per
        # issuing engine — so this emission order IS the transfer order, and
        # it is exactly consumption order. The scalar engine carries no input
        # DMAs at all: a dma_start blocks the issuing engine while the ring
        # is full, and the scalar engine must be free to run the SwiGLU silu
        # ops (which drain PSUM banks) from ~13us on — parking multi-MB loads
        # on it stalls the PE behind 8 full PSUM banks.
        def sload(shape, tag, src):
            t = wpool.tile(shape, BF16, tag=tag, name=tag)
            nc.sync.dma_start(t[:], src)
            return t

        # Ring head: the first swi pair (the first chain's lhsT) goes FIRST,
        # then xs as three 2-k-tile DMAs (262KB each: big enough to stay
        # ring-bound instead of issue-bound), then the remaining pairs. The
        # first real chain starts supply-paced at ~12us (DMA completion
        # semaphores lag the last byte by >1us) and the pair stream
        # (0.98us/pair) stays ahead of consumption (2.56us/pair).
        swi_t = [sload([P, 2 * H], "swi0", swi[0])]
        xs_t = [sload([P, 2, TS], f"xs{i}", xs[:, 2 * i:2 * i + 2, :])
                for i in range(KH // 2)]
        swi_t += [sload([P, 2 * H], f"swi{ft}", swi[ft]) for ft in range(1, KI)]
        swo_t = sload([P, KI, H], "swo", swo[:])
        xe_t = sload([P, KH, m_pad], "xe", xe[:])
        wi_t = sload([P, KH, I2], "wi", wi[:])
        wo_t = sload([P, KI, H], "wo", wo[:])

        # accessors: x(kt, a, b) -> rhs AP; w*(ft|ht, kt) -> lhsT AP
        sh = dict(
            x=lambda kt, a, b: xs_t[kt // 2][:, kt % 2, a:b],
            wa=lambda ft, kt: swi_t[ft][:, kt * P:(kt + 1) * P],
            wb=lambda ft, kt: swi_t[ft][:, H + kt * P:H + (kt + 1) * P],
            wo=lambda ht, kt: swo_t[:, kt, ht * P:(ht + 1) * P],
        )
        ex = dict(
            x=lambda kt, a, b: xe_t[:, kt, a:b],
            wa=lambda ft, kt: wi_t[:, kt, ft * P:(ft + 1) * P],
            wb=lambda ft, kt: wi_t[:, kt, I + ft * P:I + (ft + 1) * P],
            wo=lambda ht, kt: wo_t[:, kt, ht * P:(ht + 1) * P],
        )

        # (accessors, y_dram, chunk_off, chunk_sz, silu_on_first)
        chunks = []
        for acc, yd, m, sfirst in ((sh, ys, TS, True), (ex, ye, m_pad, False)):
            off = 0
            for sz in _chunk_sizes(m):
                chunks.append((acc, yd, off, sz, sfirst))
                off += sz

        def emit_wi(c):
            acc, yd, off, sz, sfirst = chunks[c]
            act = apool.tile([P, KI, MAXN], BF16, tag="act", name="act")[:, :, :sz]
            for ft in range(KI):
                ps_a = psum.tile([P, MAXN], F32, tag="ps", name="ps_a")[:, :sz]
                for kt in range(KH):
                    nc.tensor.matmul(ps_a, lhsT=acc["wa"](ft, kt),
                                     rhs=acc["x"](kt, off, off + sz),
                                     start=(kt == 0), stop=(kt == KH - 1))
                ps_b = psum.tile([P, MAXN], F32, tag="ps", name="ps_b")[:, :sz]
                for kt in range(KH):
                    nc.tensor.matmul(ps_b, lhsT=acc["wb"](ft, kt),
                                     rhs=acc["x"](kt, off, off + sz),
                                     start=(kt == 0), stop=(kt == KH - 1))
                sl = spool.tile([P, MAXN], F32, tag="silu", name="sl")[:, :sz]
                ps_s, ps_m = (ps_a, ps_b) if sfirst else (ps_b, ps_a)
                if USE_SILU:
                    # act = silu(s) * m: one ACT op + one DVE mul; PSUM banks
                    # are freed one op earlier than the sigmoid+2-mul form
                    nc.scalar.activation(sl, ps_s,
                                         mybir.ActivationFunctionType.Silu)
                    nc.vector.tensor_mul(act[:, ft, :], sl, ps_m)
                else:
                    # CoreSim fallback: silu(s) = s * sigmoid(s)
                    tmp = spool.tile([P, MAXN], F32, tag="silu2",
                                     name="tmp")[:, :sz]
                    nc.scalar.activation(sl, ps_s,
                                         mybir.ActivationFunctionType.Sigmoid)
                    nc.vector.tensor_mul(tmp, sl, ps_s)
                    nc.vector.tensor_mul(act[:, ft, :], tmp, ps_m)
            return act

        def emit_wo(c, act, last=False):
            acc, yd, off, sz, sfirst = chunks[c]
            for ht in range(KH):
                ps_y = psum.tile([P, MAXN], F32, tag="ps", name="ps_y")[:, :sz]
                for kt in range(KI):
                    nc.tensor.matmul(ps_y, lhsT=acc["wo"](ht, kt),
                                     rhs=act[:, kt, :],
                                     start=(kt == 0), stop=(kt == KI - 1))
                yt = ypool.tile([P, MAXN], BF16, tag="y", name="yt")[:, :sz]
                # copy (with f32->bf16 cast) + output DMA both on the Scalar
                # engine: its HWDGE ring carries no input loads, so outputs
                # drain immediately and never contend with the input stream on
                # the sync ring. The LAST chunk's copies ride the (idle-by-
                # then) Vector engine instead — ~3x faster per copy, pulling
                # the final output DMA earlier in the serialized tail.
                if last:
                    nc.vector.tensor_copy(yt, ps_y)
                else:
                    nc.scalar.copy(yt, ps_y)
                nc.scalar.dma_start(
                    yd.rearrange("(o p) m -> p o m", p=P)[:, ht, off:off + sz], yt)

        # software pipeline: Wi(c+1) is emitted before Wo(c) so the PE always
        # has independent matmul work while ACT/DVE finish chunk c's SwiGLU.
        n = len(chunks)
        acts = [None] * n
        acts[0] = emit_wi(0)
        emit_wo(0, acts[0])
        if n > 1:
            acts[1] = emit_wi(1)
            for c in range(2, n):
                acts[c] = emit_wi(c)
                emit_wo(c - 1, acts[c - 1])
            emit_wo(n - 1, acts[n - 1], last=True)

    nc.compile()
    return nc


def _p_major(a: np.ndarray, ktiles: int) -> np.ndarray:
    """(ktiles*P, F) -> (P, ktiles, F) partition-major contiguous layout."""
    return np.ascontiguousarray(
        a.reshape(ktiles, P, a.shape[1]).transpose(1, 0, 2))


def _tile_swi(swiT):
    """(H, 2I) -> (9, P, 2H): f-tile-pair-major layout, pairs packed in the
    exact order the PE consumes them (proj f-tile ft || gate f-tile ft)."""
    FI2 = I2 // P
    t18 = swiT.reshape(KH, P, FI2, P).transpose(2, 1, 0, 3).reshape(FI2, P, H)
    return np.ascontiguousarray(np.concatenate([t18[:KI], t18[KI:]], axis=2))


def _route(x, gate_w, correction_bias):
    logits = 1.0 / (1.0 + np.exp(-(x @ gate_w.T), dtype=np.float32))  # (T, E)
    sel = logits + correction_bias[None, :]
    order = np.argsort(-sel, axis=1, kind="stable")[:, :TOPK]  # ties -> low index
    w = np.take_along_axis(logits, order, axis=1)
    w = (w / w.sum(axis=1, keepdims=True)).astype(np.float32)
    return order, w


def kernel(**inputs) -> np.ndarray:
    x = np.asarray(inputs["x"], np.float32)
    gate_w = np.asarray(inputs["gate_w"], np.float32)
    bias = np.asarray(inputs["correction_bias"], np.float32)
    Wi = np.asarray(inputs["Wi"], np.float32)
    Wo = np.asarray(inputs["Wo"], np.float32)
    shared_Wi = np.asarray(inputs["shared_Wi"], np.float32)
    shared_Wo = np.asarray(inputs["shared_Wo"], np.float32)

    order, w = _route(x, gate_w, bias)

    idx_per_e, cw_per_e = [], []
    for e in range(E):
        mask = order == e  # (T, K)
        tok = mask.any(axis=1)
        rows = np.nonzero(tok)[0]
        kpos = np.argmax(mask[rows], axis=1)
        idx_per_e.append(rows)
        cw_per_e.append(w[rows, kpos].astype(np.float32))

    mx = max(len(r) for r in idx_per_e)
    m_pad = max(64, mx + (mx & 1))  # exact capacity, kept even for alignment

    bf = ml_dtypes.bfloat16
    xT = np.ascontiguousarray(x.T).astype(bf)            # (H, T)
    swip = _tile_swi(shared_Wi.T.astype(bf))             # (9, P, 2H)
    swop = _p_major(np.ascontiguousarray(shared_Wo.T).astype(bf), KI)

    in_maps = []
    for c in range(N_CORES):
        rows = idx_per_e[c]
        xe = np.zeros((H, m_pad), bf)
        xe[:, :len(rows)] = xT[:, rows]
        in_maps.append({
            "xs": _p_major(np.ascontiguousarray(
                xT[:, c * TS:(c + 1) * TS]), KH),         # (P, 6, TS)
            "swi": swip,
            "swo": swop,
            "xe": _p_major(xe, KH),                       # (P, 6, m_pad)
            "wi": _p_major(Wi[c].astype(bf), KH),         # (P, 6, 2I)
            "wo": _p_major(Wo[c].astype(bf), KI),         # (P, 9, H)
        })

    if m_pad not in _BUILD_CACHE:
        _BUILD_CACHE[m_pad] = _build(m_pad)
    nc = _BUILD_CACHE[m_pad]

    _ensure_axon_ntff_hook()
    res = run_bass_kernel_spmd(nc, in_maps, list(range(N_CORES)))
    global LAST_RESULTS
    LAST_RESULTS = res

    out = np.zeros((T, H), np.float32)
    for c in range(N_CORES):
        r = res.results[c]
        out[c * TS:(c + 1) * TS] += r["ys"].T.astype(np.float32)
        rows = idx_per_e[c]
        if len(rows):
            out[rows] += (r["ye"][:, :len(rows)].T.astype(np.float32)
                          * cw_per_e[c][:, None])
    return out
